# revision 39
# baseline (speedup 1.0000x reference)
"""LinOSS layer Trainium2 kernel.

Math: the per-state 2x2 recurrence matrix M = [[1, -sA], [s, 1-s^2 A]] has
det(M)=1 and eigenvalues e^{+-i theta} with cos(theta) = 1 - s^2 A / 2, so the
scanned state collapses to a rank-2 modulated prefix sum:

    u_t   = s * Bu_t            (s folded into B on host)
    T1    = gamma*cos(t th) + sin(t th);  T2 = cos(t th) - gamma*sin(t th)
    E     = cumsum(T1 * u);     F = cumsum(T2 * u)
    x_t   = sin(t th) * E_t + cos(t th) * F_t
    gamma = (s - s^2 A / 2) / sin(theta)

Sharding: states P=256 split across 8 cores (32 each); inside a core, time
L=8192 is folded 4x into partitions -> tiles are (128=[4 chunks x 32 states],
2048).  Fold-chunk carries are fixed with per-partition offsets folded into
the demodulation bias; the offsets come from row sums that the modulation
computes for free (tensor_tensor_reduce accum), so they are ready before the
scans finish.  Each core emits a partial (H, L) bf16 output; the host sums
partials, adds input*D, and transposes - the unshard step.

Device-side structure per core (driven by measured HW behavior):
  - input arrives HOST-pretransposed in jt-major layout (no DMA transpose)
  - the DVE scan runs at 2 cycles/col regardless of dtype and DVE+Pool
    contend for SBUF ports, so ALL elementwise work runs on DVE (bf16 2x
    mode) and Pool stays idle; Act (separate ports) does psum evacuation
  - the 2048-col scans are split into two INDEPENDENT 1024-col scans
    (no chaining); the second chunk's missing prefix is folded into its
    demod bias (offs + first-chunk finals)
  - x = t1 + t2 is absorbed into the projection via PSUM accumulation;
    projection matmuls of neighboring fold-chunks are interleaved so their
    accumulation chains can overlap on the PE
  - all tables are host-precomputed fp64 -> bf16
"""

import numpy as np

L, H, P = 8192, 128, 256
NCORES = 8
SLOC = P // NCORES          # states per core
FOLD = 4                    # time chunks folded into partitions
CL = L // FOLD              # 2048 free columns per partition row
NPART = FOLD * SLOC         # 128
JT = 512                    # j-tile width (psum bank)
NJT = CL // JT              # 4
SCH = 1024                  # scan chunk (2 j-tiles)
CL2 = 2 * CL

_CACHE: dict = {}


def _build_bass(split_waits=True):
    import concourse.bass as bass
    import concourse.mybir as mybir
    import concourse.tile as tile

    dt = mybir.dt.float32
    bt = mybir.dt.bfloat16
    Alu = mybir.AluOpType

    nc = bass.Bass(
        trn_type="TRN2",
        target_bir_lowering=False,
        debug=False,
        num_devices=NCORES,
    )

    inpT_d = nc.dram_tensor("inpT", [NPART, L], bt, kind="ExternalInput").ap()
    Bt_d = nc.dram_tensor("Bt", [H, 2 * SLOC], bt, kind="ExternalInput").ap()
    T1_d = nc.dram_tensor("T1", [NPART, CL], bt, kind="ExternalInput").ap()
    T2_d = nc.dram_tensor("T2", [NPART, CL], bt, kind="ExternalInput").ap()
    Sd_d = nc.dram_tensor("Sd", [NPART, CL], bt, kind="ExternalInput").ap()
    Cd_d = nc.dram_tensor("Cd", [NPART, CL], bt, kind="ExternalInput").ap()
    Ctr_d = nc.dram_tensor("Ctr", [NPART, H], bt, kind="ExternalInput").ap()
    Cti_d = nc.dram_tensor("Cti", [NPART, H], bt, kind="ExternalInput").ap()
    Wm_d = nc.dram_tensor("Wm", [NPART, NPART], bt, kind="ExternalInput").ap()
    outp = nc.dram_tensor("outp", [H, L], bt, kind="ExternalOutput").ap()

    with tile.TileContext(nc) as tc:
        cpool = tc.alloc_tile_pool(name="const", bufs=1)
        big = tc.alloc_tile_pool(name="big", bufs=1)
        evac = tc.alloc_tile_pool(name="evac", bufs=3)
        osbp = tc.alloc_tile_pool(name="osbp", bufs=8)
        psum_bu = tc.alloc_tile_pool(name="psum_bu", bufs=2, space="PSUM")
        psum_o = tc.alloc_tile_pool(name="psum_o", bufs=2, space="PSUM")

        # Few, big dma_starts: SP-side descriptor generation costs ~0.6us per
        # dma_start, so splitting fine-grained strangles the whole pipeline.
        # Two pieces per 512KB block (2 queues each), early blocks issued
        # from the Act engine (also HWDGE) so they don't queue behind SP.
        inpT = big.tile([NPART, L], bt, tag="inpT")
        T1 = big.tile([NPART, CL], bt, tag="T1")
        T2 = big.tile([NPART, CL], bt, tag="T2")
        Sd = big.tile([NPART, CL], bt, tag="Sd")
        Cd = big.tile([NPART, CL], bt, tag="Cd")

        def dma2(eng, sbuf, dram, lo, n=2048):
            for p in (0, 64):
                eng.dma_start(
                    out=sbuf[p : p + 64, lo : lo + n],
                    in_=dram[p : p + 64, lo : lo + n],
                )

        Bt = cpool.tile([H, 2 * SLOC], bt)
        nc.sync.dma_start(out=Bt[:], in_=Bt_d)
        dma2(nc.sync, inpT, inpT_d, 0)
        dma2(nc.sync, T1, T1_d, 0)
        dma2(nc.sync, T2, T2_d, 0)
        dma2(nc.sync, inpT, inpT_d, 2048)
        dma2(nc.sync, inpT, inpT_d, 4096)
        dma2(nc.sync, inpT, inpT_d, 6144)
        dma2(nc.sync, Sd, Sd_d, 0)
        dma2(nc.sync, Cd, Cd_d, 0)
        Ctr = cpool.tile_from(Ctr_d)
        Cti = cpool.tile_from(Cti_d)
        Wm = cpool.tile_from(Wm_d)

        ones = cpool.tile([NPART, SCH], bt)
        nc.gpsimd.memset(ones[:], 1.0)

        fins = cpool.tile([NPART, 4], bt)
        Y1 = big.tile([NPART, CL2], bt, tag="Y1")   # (T1*u_r | T1*u_i)
        Y2 = big.tile([NPART, CL2], bt, tag="Y2")   # (T2*u_r | T2*u_i)
        E1 = big.tile([NPART, CL2], bt, tag="E1")   # (Er | Ei)
        E2 = big.tile([NPART, CL2], bt, tag="E2")   # (Fr | Fi)

        def scan(arr, y, s):
            init = 0.0 if s.start % CL == 0 else arr[:, s.start - 1 : s.start]
            bass.BassGpSimd.tensor_tensor_scan(
                nc.vector, arr[:, s], ones[:, 0 : s.stop - s.start], y[:, s],
                init, Alu.mult, Alu.add,
            )

        # ---- Bu matmuls + modulation (rowsum accum -> early offsets) ----
        for jt in range(NJT):
            pbu = psum_bu.tile([NPART, 2 * JT], dt, tag="bu")
            for c in range(FOLD):
                rhs = inpT[:, jt * CL + c * JT : jt * CL + (c + 1) * JT]
                ps = slice(c * SLOC, (c + 1) * SLOC)
                nc.tensor.matmul(
                    pbu[ps, 0:JT], Bt[:, 0:SLOC], rhs, start=True, stop=True,
                    tile_position=(0, c * SLOC),
                )
                nc.tensor.matmul(
                    pbu[ps, JT : 2 * JT], Bt[:, SLOC : 2 * SLOC], rhs,
                    start=True, stop=True,
                    tile_position=(0, c * SLOC),
                )
            U = evac.tile([NPART, 2 * JT], bt, tag="U")
            nc.scalar.copy(U[:], pbu[:])
            js = slice(jt * JT, (jt + 1) * JT)
            jsi = slice(CL + jt * JT, CL + (jt + 1) * JT)
            Uv = U[:].rearrange("p (two j) -> p two j", two=2)
            for dst, tab in ((Y1, T1), (Y2, T2)):
                dv = dst[:].rearrange("p (two cl) -> p two cl", two=2)[:, :, js]
                tv = tab[:, js].unsqueeze(1).broadcast_to([NPART, 2, JT])
                nc.vector.tensor_mul(dv, Uv, tv)
            if jt % 2 == 1:
                ch = jt // 2
                for i, (arr, y, lo) in enumerate(
                    ((E1, Y1, 0), (E2, Y2, 0), (E1, Y1, CL), (E2, Y2, CL))
                ):
                    scan(arr, y, slice(lo + ch * SCH, lo + (ch + 1) * SCH))
                    if jt == NJT - 1:
                        col = lo + CL
                        nc.scalar.copy(
                            fins[:, i : i + 1], arr[:, col - 1 : col]
                        )

        # ---- carry offsets, real part first so Act biases can lead ----
        poffA = psum_bu.tile([NPART, 2], dt, tag="bu")
        nc.tensor.matmul(poffA[:], Wm[:], fins[:, 0:2], start=True, stop=True)
        offsA = cpool.tile([NPART, 2], dt)
        nc.scalar.copy(offsA[:], poffA[:])
        poffB = psum_bu.tile([NPART, 2], dt, tag="bu")
        nc.tensor.matmul(poffB[:], Wm[:], fins[:, 2:4], start=True, stop=True)
        offsB = cpool.tile([NPART, 2], dt)
        nc.scalar.copy(offsB[:], poffB[:])

        # ---- demod ch0 -> proj ch0 -> scans ch1 -> demod ch1 -> proj ch1 --
        t1 = big.tile([NPART, CL2], bt, tag="t1")
        t2 = big.tile([NPART, CL2], bt, tag="t2")
        xx = big.tile([NPART, CL2], bt, tag="xx")
        eE = big.tile([NPART, CL2], bt, tag="eE")
        eF = big.tile([NPART, CL2], bt, tag="eF")
        Ident = mybir.ActivationFunctionType.Identity

        def demod(ch):
            sc = slice(ch * SCH, (ch + 1) * SCH)
            sci = slice(CL + ch * SCH, CL + (ch + 1) * SCH)
            # bias on Act (separate ports), multiply on DVE (2x mode); the
            # t1 + t2 sum is absorbed into the projection's PSUM accumulation
            nc.scalar.activation(eE[:, sc], E1[:, sc], Ident, bias=offsA[:, 0:1])
            nc.vector.tensor_mul(t1[:, sc], eE[:, sc], Sd[:, sc])
            nc.scalar.activation(eF[:, sc], E2[:, sc], Ident, bias=offsA[:, 1:2])
            nc.vector.tensor_mul(t2[:, sc], eF[:, sc], Cd[:, sc])
            nc.vector.tensor_add(xx[:, sc], t1[:, sc], t2[:, sc])
            nc.scalar.activation(eE[:, sci], E1[:, sci], Ident, bias=offsB[:, 0:1])
            nc.vector.tensor_mul(t1[:, sci], eE[:, sci], Sd[:, sc])
            nc.scalar.activation(eF[:, sci], E2[:, sci], Ident, bias=offsB[:, 1:2])
            nc.vector.tensor_mul(t2[:, sci], eF[:, sci], Cd[:, sc])
            nc.vector.tensor_add(xx[:, sci], t1[:, sci], t2[:, sci])

        def proj(ch):
            base = 2 * ch * JT
            pos = []
            for c in range(FOLD):
                pool = psum_o if c < 2 else psum_bu
                po = pool.tile([NPART, 2 * JT], dt, tag="out" if c < 2 else "bu")
                pos.append(po)
            for half in range(2):
                js = slice(base + half * JT, base + (half + 1) * JT)
                jsi = slice(CL + js.start, CL + js.stop)
                pd = slice(half * JT, (half + 1) * JT)
                for k, (mat, msl) in enumerate(((Ctr, js), (Cti, jsi))):
                    for c in range(FOLD):
                        ps = slice(c * SLOC, (c + 1) * SLOC)
                        nc.tensor.matmul(
                            pos[c][:, pd], mat[ps, :], xx[ps, msl],
                            start=(k == 0),
                            stop=(k == 1),
                            tile_position=(c * SLOC, 0),
                        )
            for c in range(FOLD):
                osb = osbp.tile([NPART, 2 * JT], bt, tag="osb")
                if c < 2:
                    nc.scalar.copy(osb[:], pos[c][:])
                else:
                    nc.vector.tensor_scalar_add(osb[:], pos[c][:], 0.0)
                nc.sync.dma_start(
                    out=outp[:, c * CL + base : c * CL + base + 2 * JT],
                    in_=osb[:],
                )

        demod(0)
        demod(1)
        proj(0)
        proj(1)

        for p in (psum_o, psum_bu, osbp, evac, big, cpool):
            p.release()
    if split_waits:
        _split_matmul_waits(nc, mybir)
    return nc


def _split_matmul_waits(nc, mybir):
    """Hardware instruction structs fit a limited number of embedded sync
    waits (1 for the fp32 self-loading LDWEIGHTS matmul, 2 for ACT/DVE/POOL
    compute structs); move extra waits onto an inserted same-queue no-op."""
    caps = {"InstMatmult": 1}
    skip = {"InstNoOp", "InstAllEngineBarrier", "InstSync"}
    k = 0
    for bb in nc.main_func.blocks:
        insts = bb.instructions
        i = 0
        while i < len(insts):
            ins = insts[i]
            tn = type(ins).__name__
            if tn not in skip and ins.sync_info is not None:
                cap = caps.get(tn, 1)
                w = list(ins.sync_info.on_wait or [])
                if len(w) > cap:
                    for wj in w[:-cap]:
                        nop = mybir.InstNoOp(
                            name=f"I-mmdep-{k}",
                            engine=ins.engine,
                            ins=[],
                            outs=[],
                            sync_info=mybir.SyncInfo(
                                on_wait=[wj], on_update=[]
                            ),
                        )
                        k += 1
                        insts.insert(i, nop)
                        i += 1
                    ins.sync_info = mybir.SyncInfo(
                        on_wait=w[-cap:], on_update=ins.sync_info.on_update
                    )
            i += 1


def _host_prep(inputs):
    import ml_dtypes
    bf16 = ml_dtypes.bfloat16
    inp = np.asarray(inputs["input_sequence"], np.float32)
    # host pre-transpose into jt-major layout: SBUF col jt*CL + c*JT + j
    # holds natural time t = c*CL + jt*JT + j
    inpT = np.ascontiguousarray(
        inp.T.reshape(H, FOLD, NJT, JT).transpose(0, 2, 1, 3).reshape(H, L)
    ).astype(bf16)
    A = np.maximum(np.asarray(inputs["A_diag_raw"], np.float64), 0.0)
    s = 1.0 / (1.0 + np.exp(-np.asarray(inputs["steps_raw"], np.float64)))
    Br = np.asarray(inputs["B_real"], np.float64)
    Bi = np.asarray(inputs["B_img"], np.float64)
    Cr = np.asarray(inputs["C_real"], np.float64)
    Ci = np.asarray(inputs["C_img"], np.float64)

    costh = 1.0 - s * s * A / 2.0
    sinth = np.sqrt(np.maximum(1.0 - costh * costh, 1e-300))
    theta = np.arctan2(sinth, costh)
    gamma = (s - s * s * A / 2.0) / sinth

    twopi = 2.0 * np.pi
    q = np.arange(NPART)
    Wm_f = ((q[:, None] % SLOC == q[None, :] % SLOC)
            & (q[:, None] // SLOC < q[None, :] // SLOC)).astype(bf16)
    tbase = np.repeat(np.arange(FOLD) * CL, SLOC).astype(np.float64)
    j = np.arange(CL, dtype=np.float64)

    in_maps = []
    for k in range(NCORES):
        sl = slice(k * SLOC, (k + 1) * SLOC)
        Bt = np.empty((H, 2 * SLOC), bf16)
        Bt[:, 0:SLOC] = (s[sl, None] * Br[sl]).T.astype(bf16)
        Bt[:, SLOC:] = (s[sl, None] * Bi[sl]).T.astype(bf16)
        Ctr = np.tile(Cr[:, sl].T, (FOLD, 1)).astype(bf16)
        Cti = np.tile(-Ci[:, sl].T, (FOLD, 1)).astype(bf16)

        th_q = np.tile(theta[sl], FOLD)          # (NPART,)
        g_q = np.tile(gamma[sl], FOLD)[:, None]  # (NPART, 1)
        ang = np.mod((tbase[:, None] + j[None, :]) * th_q[:, None], twopi)
        sinT = np.sin(ang)
        cosT = np.cos(ang)
        in_maps.append({
            "inpT": inpT,
            "Bt": Bt,
            "T1": (g_q * cosT + sinT).astype(bf16),
            "T2": (cosT - g_q * sinT).astype(bf16),
            "Sd": sinT.astype(bf16),
            "Cd": cosT.astype(bf16),
            "Ctr": Ctr,
            "Cti": Cti,
            "Wm": Wm_f,
        })
    return in_maps


LAST_RESULTS = None


def kernel(**inputs) -> np.ndarray:
    global LAST_RESULTS
    from concourse.bass_utils import run_bass_kernel_spmd

    if "nc" not in _CACHE:
        _CACHE["nc"] = _build_bass()
    nc = _CACHE["nc"]

    in_maps = _host_prep(inputs)
    res = run_bass_kernel_spmd(nc, in_maps, core_ids=list(range(NCORES)))
    LAST_RESULTS = res
    part = np.zeros((H, L), np.float32)
    for r in res.results:
        part += r["outp"].astype(np.float32)
    out = part.T + np.asarray(inputs["input_sequence"], np.float32) * np.asarray(
        inputs["D"], np.float32
    )
    return np.ascontiguousarray(out)


# revision 40
# speedup vs baseline: 1.0309x; 1.0309x over previous
"""LinOSS layer Trainium2 kernel.

Math: the per-state 2x2 recurrence matrix M = [[1, -sA], [s, 1-s^2 A]] has
det(M)=1 and eigenvalues e^{+-i theta} with cos(theta) = 1 - s^2 A / 2, so the
scanned state collapses to a rank-2 modulated prefix sum:

    u_t   = s * Bu_t            (s folded into B on host)
    T1    = gamma*cos(t th) + sin(t th);  T2 = cos(t th) - gamma*sin(t th)
    E     = cumsum(T1 * u);     F = cumsum(T2 * u)
    x_t   = sin(t th) * E_t + cos(t th) * F_t
    gamma = (s - s^2 A / 2) / sin(theta)

Sharding: states P=256 split across 8 cores (32 each); inside a core, time
L=8192 is folded 4x into partitions -> tiles are (128=[4 chunks x 32 states],
2048).  Fold-chunk carries are fixed with per-partition offsets folded into
the demodulation bias; the offsets come from row sums that the modulation
computes for free (tensor_tensor_reduce accum), so they are ready before the
scans finish.  Each core emits a partial (H, L) bf16 output; the host sums
partials, adds input*D, and transposes - the unshard step.

Device-side structure per core (driven by measured HW behavior):
  - input arrives HOST-pretransposed in jt-major layout (no DMA transpose)
  - the DVE scan runs at 2 cycles/col regardless of dtype and DVE+Pool
    contend for SBUF ports, so ALL elementwise work runs on DVE (bf16 2x
    mode) and Pool stays idle; Act (separate ports) does psum evacuation
  - the 2048-col scans are split into two INDEPENDENT 1024-col scans
    (no chaining); the second chunk's missing prefix is folded into its
    demod bias (offs + first-chunk finals)
  - x = t1 + t2 is absorbed into the projection via PSUM accumulation;
    projection matmuls of neighboring fold-chunks are interleaved so their
    accumulation chains can overlap on the PE
  - all tables are host-precomputed fp64 -> bf16
"""

import numpy as np

L, H, P = 8192, 128, 256
NCORES = 8
SLOC = P // NCORES          # states per core
FOLD = 4                    # time chunks folded into partitions
CL = L // FOLD              # 2048 free columns per partition row
NPART = FOLD * SLOC         # 128
JT = 512                    # j-tile width (psum bank)
NJT = CL // JT              # 4
SCH = 1024                  # scan chunk (2 j-tiles)
CL2 = 2 * CL

_CACHE: dict = {}


def _build_bass(split_waits=True):
    import concourse.bass as bass
    import concourse.mybir as mybir
    import concourse.tile as tile

    dt = mybir.dt.float32
    bt = mybir.dt.bfloat16
    Alu = mybir.AluOpType

    nc = bass.Bass(
        trn_type="TRN2",
        target_bir_lowering=False,
        debug=False,
        num_devices=NCORES,
    )

    inpT_d = nc.dram_tensor("inpT", [NPART, L], bt, kind="ExternalInput").ap()
    Bt_d = nc.dram_tensor("Bt", [H, 2 * SLOC], bt, kind="ExternalInput").ap()
    T1_d = nc.dram_tensor("T1", [NPART, CL], bt, kind="ExternalInput").ap()
    T2_d = nc.dram_tensor("T2", [NPART, CL], bt, kind="ExternalInput").ap()
    Sd_d = nc.dram_tensor("Sd", [NPART, CL], bt, kind="ExternalInput").ap()
    Cd_d = nc.dram_tensor("Cd", [NPART, CL], bt, kind="ExternalInput").ap()
    Ctr_d = nc.dram_tensor("Ctr", [NPART, H], bt, kind="ExternalInput").ap()
    Cti_d = nc.dram_tensor("Cti", [NPART, H], bt, kind="ExternalInput").ap()
    Wm_d = nc.dram_tensor("Wm", [NPART, NPART], bt, kind="ExternalInput").ap()
    outp = nc.dram_tensor("outp", [H, L], bt, kind="ExternalOutput").ap()

    with tile.TileContext(nc) as tc:
        cpool = tc.alloc_tile_pool(name="const", bufs=1)
        big = tc.alloc_tile_pool(name="big", bufs=1)
        evac = tc.alloc_tile_pool(name="evac", bufs=3)
        osbp = tc.alloc_tile_pool(name="osbp", bufs=8)
        psum_bu = tc.alloc_tile_pool(name="psum_bu", bufs=2, space="PSUM")
        psum_o = tc.alloc_tile_pool(name="psum_o", bufs=2, space="PSUM")

        # Few, big dma_starts: SP-side descriptor generation costs ~0.6us per
        # dma_start, so splitting fine-grained strangles the whole pipeline.
        # Two pieces per 512KB block (2 queues each), early blocks issued
        # from the Act engine (also HWDGE) so they don't queue behind SP.
        inpT = big.tile([NPART, L], bt, tag="inpT")
        T1 = big.tile([NPART, CL], bt, tag="T1")
        T2 = big.tile([NPART, CL], bt, tag="T2")
        Sd = big.tile([NPART, CL], bt, tag="Sd")
        Cd = big.tile([NPART, CL], bt, tag="Cd")

        def dma2(eng, sbuf, dram, lo, n=2048):
            for p in (0, 64):
                eng.dma_start(
                    out=sbuf[p : p + 64, lo : lo + n],
                    in_=dram[p : p + 64, lo : lo + n],
                )

        Bt = cpool.tile([H, 2 * SLOC], bt)
        nc.sync.dma_start(out=Bt[:], in_=Bt_d)
        dma2(nc.sync, inpT, inpT_d, 0)
        dma2(nc.sync, T1, T1_d, 0)
        dma2(nc.sync, T2, T2_d, 0)
        dma2(nc.sync, inpT, inpT_d, 2048)
        dma2(nc.sync, inpT, inpT_d, 4096)
        dma2(nc.sync, inpT, inpT_d, 6144)
        dma2(nc.sync, Sd, Sd_d, 0)
        dma2(nc.sync, Cd, Cd_d, 0)
        Ctr = cpool.tile_from(Ctr_d)
        Cti = cpool.tile_from(Cti_d)
        Wm = cpool.tile_from(Wm_d)

        ones = cpool.tile([NPART, SCH], bt)
        nc.gpsimd.memset(ones[:], 1.0)

        fins = cpool.tile([NPART, 4], bt)
        Y1 = big.tile([NPART, CL2], bt, tag="Y1")   # (T1*u_r | T1*u_i)
        Y2 = big.tile([NPART, CL2], bt, tag="Y2")   # (T2*u_r | T2*u_i)
        E1 = big.tile([NPART, CL2], bt, tag="E1")   # (Er | Ei)
        E2 = big.tile([NPART, CL2], bt, tag="E2")   # (Fr | Fi)

        def scan(arr, y, s):
            init = 0.0 if s.start % CL == 0 else arr[:, s.start - 1 : s.start]
            bass.BassGpSimd.tensor_tensor_scan(
                nc.vector, arr[:, s], ones[:, 0 : s.stop - s.start], y[:, s],
                init, Alu.mult, Alu.add,
            )

        # ---- Bu matmuls + modulation (rowsum accum -> early offsets) ----
        for jt in range(NJT):
            pbu = psum_bu.tile([NPART, 2 * JT], dt, tag="bu")
            for c in range(FOLD):
                rhs = inpT[:, jt * CL + c * JT : jt * CL + (c + 1) * JT]
                ps = slice(c * SLOC, (c + 1) * SLOC)
                nc.tensor.matmul(
                    pbu[ps, 0:JT], Bt[:, 0:SLOC], rhs, start=True, stop=True,
                    tile_position=(0, c * SLOC),
                )
                nc.tensor.matmul(
                    pbu[ps, JT : 2 * JT], Bt[:, SLOC : 2 * SLOC], rhs,
                    start=True, stop=True,
                    tile_position=(0, c * SLOC),
                )
            U = evac.tile([NPART, 2 * JT], bt, tag="U")
            nc.scalar.copy(U[:], pbu[:])
            js = slice(jt * JT, (jt + 1) * JT)
            jsi = slice(CL + jt * JT, CL + (jt + 1) * JT)
            Uv = U[:].rearrange("p (two j) -> p two j", two=2)
            for dst, tab in ((Y1, T1), (Y2, T2)):
                dv = dst[:].rearrange("p (two cl) -> p two cl", two=2)[:, :, js]
                tv = tab[:, js].unsqueeze(1).broadcast_to([NPART, 2, JT])
                nc.vector.tensor_mul(dv, Uv, tv)
            for i, (arr, y, sl) in enumerate(
                ((E1, Y1, js), (E2, Y2, js), (E1, Y1, jsi), (E2, Y2, jsi))
            ):
                scan(arr, y, sl)
                if jt == NJT - 1:
                    col = sl.stop
                    nc.scalar.copy(fins[:, i : i + 1], arr[:, col - 1 : col])

        # ---- carry offsets, real part first so Act biases can lead ----
        poffA = psum_bu.tile([NPART, 2], dt, tag="bu")
        nc.tensor.matmul(poffA[:], Wm[:], fins[:, 0:2], start=True, stop=True)
        offsA = cpool.tile([NPART, 2], dt)
        nc.scalar.copy(offsA[:], poffA[:])
        poffB = psum_bu.tile([NPART, 2], dt, tag="bu")
        nc.tensor.matmul(poffB[:], Wm[:], fins[:, 2:4], start=True, stop=True)
        offsB = cpool.tile([NPART, 2], dt)
        nc.scalar.copy(offsB[:], poffB[:])

        # ---- demod ch0 -> proj ch0 -> scans ch1 -> demod ch1 -> proj ch1 --
        t1 = big.tile([NPART, CL2], bt, tag="t1")
        t2 = big.tile([NPART, CL2], bt, tag="t2")
        xx = big.tile([NPART, CL2], bt, tag="xx")
        eE = big.tile([NPART, CL2], bt, tag="eE")
        eF = big.tile([NPART, CL2], bt, tag="eF")
        Ident = mybir.ActivationFunctionType.Identity

        def demod(ch):
            sc = slice(ch * SCH, (ch + 1) * SCH)
            sci = slice(CL + ch * SCH, CL + (ch + 1) * SCH)
            # bias on Act (separate ports), multiply on DVE (2x mode); the
            # t1 + t2 sum is absorbed into the projection's PSUM accumulation
            nc.scalar.activation(eE[:, sc], E1[:, sc], Ident, bias=offsA[:, 0:1])
            nc.vector.tensor_mul(t1[:, sc], eE[:, sc], Sd[:, sc])
            nc.scalar.activation(eF[:, sc], E2[:, sc], Ident, bias=offsA[:, 1:2])
            nc.vector.tensor_mul(t2[:, sc], eF[:, sc], Cd[:, sc])
            nc.vector.tensor_add(xx[:, sc], t1[:, sc], t2[:, sc])
            nc.scalar.activation(eE[:, sci], E1[:, sci], Ident, bias=offsB[:, 0:1])
            nc.vector.tensor_mul(t1[:, sci], eE[:, sci], Sd[:, sc])
            nc.scalar.activation(eF[:, sci], E2[:, sci], Ident, bias=offsB[:, 1:2])
            nc.vector.tensor_mul(t2[:, sci], eF[:, sci], Cd[:, sc])
            nc.vector.tensor_add(xx[:, sci], t1[:, sci], t2[:, sci])

        def proj(ch):
            base = 2 * ch * JT
            pos = []
            for c in range(FOLD):
                pool = psum_o if c < 2 else psum_bu
                po = pool.tile([NPART, 2 * JT], dt, tag="out" if c < 2 else "bu")
                pos.append(po)
            for half in range(2):
                js = slice(base + half * JT, base + (half + 1) * JT)
                jsi = slice(CL + js.start, CL + js.stop)
                pd = slice(half * JT, (half + 1) * JT)
                for k, (mat, msl) in enumerate(((Ctr, js), (Cti, jsi))):
                    for c in range(FOLD):
                        ps = slice(c * SLOC, (c + 1) * SLOC)
                        nc.tensor.matmul(
                            pos[c][:, pd], mat[ps, :], xx[ps, msl],
                            start=(k == 0),
                            stop=(k == 1),
                            tile_position=(c * SLOC, 0),
                        )
            for c in range(FOLD):
                osb = osbp.tile([NPART, 2 * JT], bt, tag="osb")
                if c < 2:
                    nc.scalar.copy(osb[:], pos[c][:])
                else:
                    nc.vector.tensor_scalar_add(osb[:], pos[c][:], 0.0)
                nc.sync.dma_start(
                    out=outp[:, c * CL + base : c * CL + base + 2 * JT],
                    in_=osb[:],
                )

        demod(0)
        demod(1)
        proj(0)
        proj(1)

        for p in (psum_o, psum_bu, osbp, evac, big, cpool):
            p.release()
    if split_waits:
        _split_matmul_waits(nc, mybir)
    return nc


def _split_matmul_waits(nc, mybir):
    """Hardware instruction structs fit a limited number of embedded sync
    waits (1 for the fp32 self-loading LDWEIGHTS matmul, 2 for ACT/DVE/POOL
    compute structs); move extra waits onto an inserted same-queue no-op."""
    caps = {"InstMatmult": 1}
    skip = {"InstNoOp", "InstAllEngineBarrier", "InstSync"}
    k = 0
    for bb in nc.main_func.blocks:
        insts = bb.instructions
        i = 0
        while i < len(insts):
            ins = insts[i]
            tn = type(ins).__name__
            if tn not in skip and ins.sync_info is not None:
                cap = caps.get(tn, 1)
                w = list(ins.sync_info.on_wait or [])
                if len(w) > cap:
                    for wj in w[:-cap]:
                        nop = mybir.InstNoOp(
                            name=f"I-mmdep-{k}",
                            engine=ins.engine,
                            ins=[],
                            outs=[],
                            sync_info=mybir.SyncInfo(
                                on_wait=[wj], on_update=[]
                            ),
                        )
                        k += 1
                        insts.insert(i, nop)
                        i += 1
                    ins.sync_info = mybir.SyncInfo(
                        on_wait=w[-cap:], on_update=ins.sync_info.on_update
                    )
            i += 1


def _host_prep(inputs):
    import ml_dtypes
    bf16 = ml_dtypes.bfloat16
    inp = np.asarray(inputs["input_sequence"], np.float32)
    # host pre-transpose into jt-major layout: SBUF col jt*CL + c*JT + j
    # holds natural time t = c*CL + jt*JT + j
    inpT = np.ascontiguousarray(
        inp.T.reshape(H, FOLD, NJT, JT).transpose(0, 2, 1, 3).reshape(H, L)
    ).astype(bf16)
    A = np.maximum(np.asarray(inputs["A_diag_raw"], np.float64), 0.0)
    s = 1.0 / (1.0 + np.exp(-np.asarray(inputs["steps_raw"], np.float64)))
    Br = np.asarray(inputs["B_real"], np.float64)
    Bi = np.asarray(inputs["B_img"], np.float64)
    Cr = np.asarray(inputs["C_real"], np.float64)
    Ci = np.asarray(inputs["C_img"], np.float64)

    costh = 1.0 - s * s * A / 2.0
    sinth = np.sqrt(np.maximum(1.0 - costh * costh, 1e-300))
    theta = np.arctan2(sinth, costh)
    gamma = (s - s * s * A / 2.0) / sinth

    twopi = 2.0 * np.pi
    q = np.arange(NPART)
    Wm_f = ((q[:, None] % SLOC == q[None, :] % SLOC)
            & (q[:, None] // SLOC < q[None, :] // SLOC)).astype(bf16)
    tbase = np.repeat(np.arange(FOLD) * CL, SLOC).astype(np.float64)
    j = np.arange(CL, dtype=np.float64)

    in_maps = []
    for k in range(NCORES):
        sl = slice(k * SLOC, (k + 1) * SLOC)
        Bt = np.empty((H, 2 * SLOC), bf16)
        Bt[:, 0:SLOC] = (s[sl, None] * Br[sl]).T.astype(bf16)
        Bt[:, SLOC:] = (s[sl, None] * Bi[sl]).T.astype(bf16)
        Ctr = np.tile(Cr[:, sl].T, (FOLD, 1)).astype(bf16)
        Cti = np.tile(-Ci[:, sl].T, (FOLD, 1)).astype(bf16)

        th_q = np.tile(theta[sl], FOLD)          # (NPART,)
        g_q = np.tile(gamma[sl], FOLD)[:, None]  # (NPART, 1)
        ang = np.mod((tbase[:, None] + j[None, :]) * th_q[:, None], twopi)
        sinT = np.sin(ang)
        cosT = np.cos(ang)
        in_maps.append({
            "inpT": inpT,
            "Bt": Bt,
            "T1": (g_q * cosT + sinT).astype(bf16),
            "T2": (cosT - g_q * sinT).astype(bf16),
            "Sd": sinT.astype(bf16),
            "Cd": cosT.astype(bf16),
            "Ctr": Ctr,
            "Cti": Cti,
            "Wm": Wm_f,
        })
    return in_maps


LAST_RESULTS = None


def kernel(**inputs) -> np.ndarray:
    global LAST_RESULTS
    from concourse.bass_utils import run_bass_kernel_spmd

    if "nc" not in _CACHE:
        _CACHE["nc"] = _build_bass()
    nc = _CACHE["nc"]

    in_maps = _host_prep(inputs)
    res = run_bass_kernel_spmd(nc, in_maps, core_ids=list(range(NCORES)))
    LAST_RESULTS = res
    part = np.zeros((H, L), np.float32)
    for r in res.results:
        part += r["outp"].astype(np.float32)
    out = part.T + np.asarray(inputs["input_sequence"], np.float32) * np.asarray(
        inputs["D"], np.float32
    )
    return np.ascontiguousarray(out)
